# revision 32
# baseline (speedup 1.0000x reference)
"""Multi-head attention (N=2, S=4096, E=512, H=8) on 8 TRN2 NeuronCores.

Sharding: sequence-parallel over (batch, q-chunk): core c handles batch
c//4, query rows (c%4)*1024 .. +1024, computing all 8 heads for those
queries locally (K/V are computed for the full 4096-key sequence of its
batch).  No collectives; the host concatenates the 8 output slices.

v3b: the attention kt-loop keeps the PE in 64x128 row-tiled mode for
every matmul (tile_position (0,0)/(64,0) only -- no column tiling, no
mode switches):
  - S: as v1 -- head h0's [128keys, 512q] scores on row-tile 0 (d 0-63
    on SBUF rows 0-63), h1 on row-tile 1; the two stream concurrently.
  - O: each head's [65,512] output (V-dims + ones-column denominator)
    is split by KEY HALF into two 64-contraction matmuls on opposite
    row tiles, accumulating into two separate psum banks (row tiles
    cannot share a bank).  The two halves stream concurrently, so the
    4 O matmuls per key tile cost ~2 column-slots instead of 2 serial
    full-contraction slots.  po_lo+po_hi are summed on GpSimd at the
    end of the kt loop (16 adds, idle engine).
  - per-kt PE work: 6 matmuls of 512 cols in 2 concurrent row streams
    = ~3 x 512-col slots (~650ns) vs v1's 4 serial slots (~873ns).
  - exp alternates ScalarE (exact, even kt) / DVE (Schraudolph, odd
    kt): 1:1 to fit the shorter period; est rel err ~1.45e-2 < 2e-2.
  - PSUM pools phase-scoped: proj pool (3 banks) released before the
    attention pool (sx 2x2 banks + o 4 banks); fc psums ride the sx
    ring so everything fits in 8 banks with fc still interleaved.
Rejected: 64x64 column tiling (sim-correct but crashes HW -- col
quadrant 3 bug), DMA-XBAR transposes (1.2us/instr serial Sync ucode),
bf16 pre-cast PE transposes (staging serialization), fp8 DoubleRow
scores (rel err 2.4e-2 > gate), DRAM AllGather K/V dedup.
"""

import math
import sys

for _p in ("/opt/trn_rl_repo", "/opt/pypackages"):
    if _p not in sys.path:
        sys.path.append(_p)

import numpy as np

import concourse.bass as bass  # noqa: F401
import concourse.mybir as mybir
import concourse.tile as tile
from concourse import bacc
from concourse.bass_utils import run_bass_kernel_spmd

P = 128
N_BATCH = 2
S = 4096
E = 512
H = 8
HD = 64
NCORES = 8
QS = 1024  # query rows per core
KTT = S // P  # 32 key tiles
SCALE = 1.0 / math.sqrt(E)
# Schraudolph exp in bf16-bit domain: bits_i16 = round(s*A + B) viewed as
# bf16 ~= exp(s*SCALE).  B bias -6 minimizes max rel err (~3.5%) on the
# observed score range |s*SCALE| <= ~2.5.
A_SCH = SCALE * 128.0 / math.log(2.0)
B_SCH = 127.0 * 128.0 - 6.0
F32 = mybir.dt.float32
BF16 = mybir.dt.bfloat16
I16 = mybir.dt.int16
Exp = mybir.ActivationFunctionType.Exp
Mult = mybir.AluOpType.mult
Add = mybir.AluOpType.add

OLAG = 4  # O matmuls run this many key tiles behind S/exp


def build_core_program():
    nc = bacc.Bacc("TRN2", target_bir_lowering=False, debug=False)

    xq = nc.dram_tensor("xq", [QS, E], F32, kind="ExternalInput").ap()
    xk = nc.dram_tensor("xk", [S, E], F32, kind="ExternalInput").ap()
    xv = nc.dram_tensor("xv", [S, E], F32, kind="ExternalInput").ap()
    w_in = {
        name: nc.dram_tensor(f"w{name}", [E, E], F32, kind="ExternalInput").ap()
        for name in ("q", "k", "v", "fc")
    }
    out = nc.dram_tensor("out", [QS, E], F32, kind="ExternalOutput").ap()

    from contextlib import ExitStack

    with tile.TileContext(nc) as tc, ExitStack() as ctx:
        ep = ctx.enter_context
        ci = ep(tc.tile_pool(name="ci", bufs=16))
        co = ep(tc.tile_pool(name="co", bufs=3))
        big = ep(tc.tile_pool(name="big", bufs=1))
        xch = ep(tc.tile_pool(name="xch", bufs=4))
        atp = ep(tc.tile_pool(name="atp", bufs=6))
        small = ep(tc.tile_pool(name="small", bufs=3))

        from concourse.masks import make_identity

        ident = big.tile([P, P], F32, tag="ident")
        make_identity(nc, ident[:])

        # rotate psum->sbuf copies between DVE and ScalarE
        _cp = [0]

        def copy_cast(dst, src):
            if _cp[0] % 2 == 0:
                nc.vector.tensor_copy(dst, src)
            else:
                nc.scalar.copy(dst, src)
            _cp[0] += 1

        qT = big.tile([P, 4, QS], BF16, tag="qT")
        kT = big.tile([P, 4, S], BF16, tag="kT")
        # V with a ones column per head: [128, ktile, h*(HD+1)+d], col HD == 1.0
        Vp = big.tile([P, S // P, H * (HD + 1)], BF16, tag="Vp")
        nc.any.memset(
            Vp[:].rearrange("p k (h w) -> p k h w", w=HD + 1)[:, :, :, HD], 1.0
        )
        concatT = big.tile([P, 4, QS], BF16, tag="concatT")
        wT = {}

        # ======== phase 1: staging + projections (own PSUM pool) ========
        with tc.tile_pool(name="pp", bufs=3, space="PSUM") as pp:

            # staged transpose: one 512-row chunk of a fp32 [rows, E] input
            # -> SBUF [128, 4, 512] bf16 via PE transpose
            def stage_chunk(src, r0):
                tfs = []
                for rt in range(4):
                    tf = ci.tile([P, E], F32, tag="ci", name="tf")
                    nc.sync.dma_start(
                        tf[:], src[r0 + rt * P : r0 + (rt + 1) * P, :]
                    )
                    tfs.append(tf)
                xt = xch.tile([P, 4, 512], BF16, tag="xc", name="xt")
                for sub in range(4):
                    ps = pp.tile([P, 512], F32, tag="s", name="pst")
                    for rt in range(4):
                        nc.tensor.transpose(
                            ps[:, rt * P : (rt + 1) * P],
                            tfs[rt][:, sub * P : (sub + 1) * P],
                            ident[:],
                        )
                    copy_cast(xt[:, sub, :], ps[:])
                return xt

            # weights: resident transposed copies
            for name in w_in:
                xt = stage_chunk(w_in[name], 0)
                wt = big.tile([P, 4, E], BF16, tag=f"w{name}", name="wt")
                nc.vector.tensor_copy(wt[:], xt[:])
                wT[name] = wt

            for qc in range(QS // 512):
                xt = stage_chunk(xq, qc * 512)
                for p4 in range(4):
                    ps = pp.tile([P, 512], F32, tag="s", name="psq")
                    for sub in range(4):
                        nc.tensor.matmul(
                            ps[:],
                            lhsT=wT["q"][:, sub, p4 * P : (p4 + 1) * P],
                            rhs=xt[:, sub, :],
                            start=(sub == 0),
                            stop=(sub == 3),
                        )
                    copy_cast(qT[:, p4, qc * 512 : (qc + 1) * 512], ps[:])
            for kc in range(S // 512):
                xt = stage_chunk(xk, kc * 512)
                for p4 in range(4):
                    ps = pp.tile([P, 512], F32, tag="s", name="psk")
                    for sub in range(4):
                        nc.tensor.matmul(
                            ps[:],
                            lhsT=wT["k"][:, sub, p4 * P : (p4 + 1) * P],
                            rhs=xt[:, sub, :],
                            start=(sub == 0),
                            stop=(sub == 3),
                        )
                    copy_cast(kT[:, p4, kc * 512 : (kc + 1) * 512], ps[:])
            for kg in range(S // 512):
                xt = stage_chunk(xv, kg * 512)
                for ktl in range(4):
                    kt = kg * 4 + ktl
                    ps = pp.tile([P, 512], F32, tag="s", name="psv")
                    for sub in range(4):
                        nc.tensor.matmul(
                            ps[:],
                            lhsT=xt[:, sub, ktl * P : (ktl + 1) * P],
                            rhs=wT["v"][:, sub, :],
                            start=(sub == 0),
                            stop=(sub == 3),
                        )
                    copy_cast(
                        Vp[:, kt, :].rearrange("p (h w) -> p h w", w=HD + 1)[
                            :, :, :HD
                        ],
                        ps[:].rearrange("p (h d) -> p h d", d=HD),
                    )

        # ======== phase 2: attention + fc (own PSUM pool) ========
        with tc.tile_pool(name="pa", bufs=2, space="PSUM") as pa:

            def fc_block(qt):
                # fc psums ride the "sx" ring (use bank 0 of the 2-bank tile)
                ps2 = pa.tile([P, 2, 512], F32, tag="sx", name="psf")
                ps = ps2[:, 0, :]
                for sub in range(4):
                    nc.tensor.matmul(
                        ps,
                        lhsT=concatT[:, sub, qt * P : (qt + 1) * P],
                        rhs=wT["fc"][:, sub, :],
                        start=(sub == 0),
                        stop=(sub == 3),
                    )
                ot = co.tile([P, 512], F32, tag="of", name="ot")
                copy_cast(ot[:], ps)
                nc.sync.dma_start(out[qt * P : (qt + 1) * P, :], ot[:])

            for qc in range(QS // 512):
                for p4 in range(4):
                    po = [
                        pa.tile(
                            [HD + 1, 512],
                            F32,
                            tag="o",
                            bufs=4,
                            name=f"po{_h}",
                        )
                        for _h in range(2)
                    ]
                    ats = {}

                    def emit_S(kt, pss):
                        for h2 in range(2):
                            nc.tensor.matmul(
                                pss[:, h2, :],
                                lhsT=kT[
                                    h2 * HD : (h2 + 1) * HD,
                                    p4,
                                    kt * P : (kt + 1) * P,
                                ],
                                rhs=qT[
                                    h2 * HD : (h2 + 1) * HD,
                                    p4,
                                    qc * 512 : (qc + 1) * 512,
                                ],
                                start=True,
                                stop=True,
                                tile_position=(h2 * HD, 0),
                            )

                    def emit_O(kt):
                        at = ats.pop(kt)
                        for h2 in range(2):
                            h = p4 * 2 + h2
                            nc.tensor.matmul(
                                po[h2][:],
                                lhsT=Vp[:, kt, h * (HD + 1) : (h + 1) * (HD + 1)],
                                rhs=at[:, h2, :],
                                start=(kt == 0),
                                stop=(kt == KTT - 1),
                                skip_group_check=True,
                            )

                    for kt in range(KTT + OLAG):
                        if kt < KTT:
                            pss = pa.tile(
                                [P, 2, 512], F32, tag="sx", name="pss"
                            )
                            emit_S(kt, pss)
                            at = atp.tile(
                                [P, 2, 512], BF16, tag="at", name="at"
                            )
                            # 5:3 ACT:DVE split of the exp work
                            if kt % 8 in (0, 2, 4, 5, 7):
                                nc.scalar.activation(
                                    at[:], pss[:], Exp, scale=SCALE
                                )
                            else:
                                nc.vector.tensor_scalar(
                                    at[:].bitcast(I16),
                                    pss[:],
                                    A_SCH,
                                    B_SCH,
                                    Mult,
                                    Add,
                                )
                            ats[kt] = at
                        if kt >= OLAG:
                            emit_O(kt - OLAG)

                    for h2 in range(2):
                        # v1 normalize: stage denominator row to base-0
                        # SBUF, reciprocal, gpsimd broadcast, DVE mul
                        dn = small.tile([1, 512], F32, tag="dn")
                        nc.vector.tensor_copy(dn[:], po[h2][HD : HD + 1, :])
                        rc = small.tile([1, 512], F32, tag="rc")
                        nc.vector.reciprocal_approx_fast(rc[:], dn[:])
                        rcb = small.tile([HD, 512], F32, tag="rcb")
                        nc.gpsimd.partition_broadcast(rcb[:], rc[:])
                        nc.vector.tensor_mul(
                            concatT[
                                h2 * HD : (h2 + 1) * HD,
                                p4,
                                qc * 512 : (qc + 1) * 512,
                            ],
                            po[h2][:HD, :],
                            rcb[:],
                        )
                    # fc for the previous 512-q chunk, emitted after this
                    # qc's first block so its normalize chain never stalls
                    # the PE queue
                    if p4 == 0 and qc > 0:
                        for qt in range((qc - 1) * 4, qc * 4):
                            fc_block(qt)
            for qt in range(4, QS // P):
                fc_block(qt)

    nc.compile()
    return nc


_NC_CACHE = None


def _get_nc():
    global _NC_CACHE
    if _NC_CACHE is None:
        _NC_CACHE = build_core_program()
    return _NC_CACHE


def make_in_maps(input_v, input_q, input_k, W_Q, W_K, W_V, W_fc):
    in_maps = []
    for c in range(NCORES):
        n, qlo = c // 4, (c % 4) * QS
        in_maps.append(
            {
                "xq": np.ascontiguousarray(input_q[n, qlo : qlo + QS]),
                "xk": np.ascontiguousarray(input_k[n]),
                "xv": np.ascontiguousarray(input_v[n]),
                "wq": W_Q,
                "wk": W_K,
                "wv": W_V,
                "wfc": W_fc,
            }
        )
    return in_maps


def assemble(results):
    out = np.empty((N_BATCH, S, E), np.float32)
    for c in range(NCORES):
        n, qlo = c // 4, (c % 4) * QS
        out[n, qlo : qlo + QS] = results[c]["out"]
    return out


def kernel(input_v, input_q, input_k, W_Q, W_K, W_V, W_fc):
    args = [
        np.asarray(a, dtype=np.float32)
        for a in (input_v, input_q, input_k, W_Q, W_K, W_V, W_fc)
    ]
    nc = _get_nc()
    res = run_bass_kernel_spmd(
        nc, make_in_maps(*args), core_ids=list(range(NCORES)), trace=False
    )
    return assemble(res.results)
